# revision 1
# baseline (speedup 1.0000x reference)
"""Causal self-attention (B=4, T=2048, C=1024, H=16) on 8 TRN2 NeuronCores.

Sharding: core = (batch, head-group) — data parallel over the 4 batches,
tensor parallel over 2 groups of 8 heads (Megatron-style column/row split of
the qkv / out projections).  Each core computes a [T, C] partial of the out
projection for its head group; the host sums the two partials per batch and
adds b_out, so no device collectives are needed.

Device kernel (identical SPMD program, per-core data), interleaved per
tq-slab of 512 so projections, attention and the out-projection overlap:
  A(s) q^T,k^T = (W_qk chunk)^T @ x^T slab; v = x @ W_v (+bias via K=1 ones
       matmul).  All matmuls run as float32r (TF32-like, full fp32 storage,
       1 cycle/row).  PSUM evictions on DVE (tensor_scalar: scale+bias) so
       ScalarE runs only Exp (no activation-table reloads).
  B(h,s) S^T tiles = k^T-block @ q^T-slab (K=64; odd heads in partitions
       64-127 so even/odd pairs can overlap in distinct PE row groups), one
       Exp pass on ScalarE, causal 0/1 mask applied post-exp on GpSimd only
       for the diagonal 128x128 subtile, then P@V with lhsT=[v | 1] so the
       softmax denominator accumulates for free as PSUM row D; normalize via
       reciprocal + DRAM-bounce broadcast at eviction.
  C(s) out partial = y^T chunks @ W_out chunks.
"""

import os
import sys
from contextlib import ExitStack

import numpy as np

for _p in ("/opt/trn_rl_repo", "/root/.axon_site/_ro/trn_rl_repo"):
    if os.path.isdir(_p) and _p not in sys.path:
        sys.path.append(_p)

import concourse.bacc as bacc
import concourse.bass as bass
import concourse.tile as tile
from concourse import mybir
from concourse.bass_utils import run_bass_kernel_spmd
from concourse.masks import make_upper_triangular

AF = mybir.ActivationFunctionType
ALU = mybir.AluOpType
F32 = mybir.dt.float32
F32R = mybir.dt.float32r

P = 128
SLAB = 512

B, T, C, H, D = 4, 2048, 1024, 16, 64
N_CORES = 8
N_GROUPS = 2          # head groups (tensor-parallel degree per batch)
HL = H // N_GROUPS    # heads per core
CL = HL * D           # local qkv width


def _build_nc(loop_reps=None, parts="abc"):
    if "a" in parts:
        parts = parts.replace("a", "xqw")
    NCK = C // P
    MQK = 2 * CL // P
    MQ = MQK // 2
    TT = T // P
    NS = T // SLAB
    YC = CL // P
    W_OUT = min(SLAB, C)
    NOUT = C // W_OUT
    scale = 1.0 / np.sqrt(D)

    nc = bacc.Bacc("TRN2", target_bir_lowering=False, debug=False,
                   num_devices=N_CORES)
    xT = nc.dram_tensor("xT", [C, T], F32R, kind="ExternalInput")
    wqk = nc.dram_tensor("wqk", [C, 2 * CL], F32R, kind="ExternalInput")
    wv = nc.dram_tensor("wv", [C, CL], F32R, kind="ExternalInput")
    wout = nc.dram_tensor("wout", [CL, C], F32R, kind="ExternalInput")
    bqk = nc.dram_tensor("bqk", [P, MQK], F32, kind="ExternalInput")
    bv = nc.dram_tensor("bv", [1, CL], F32R, kind="ExternalInput")
    outp = nc.dram_tensor("outp", [T, C], F32, kind="ExternalOutput")
    scr = nc.dram_tensor("scr", [HL * NS, SLAB], F32)

    with tile.TileContext(nc) as tc, ExitStack() as ctx:
        pool = lambda name, bufs, **kw: ctx.enter_context(
            tc.tile_pool(name=name, bufs=bufs, **kw))

        const = pool("const", 1)
        kp = pool("kp", 1)
        vp = pool("vp", 1)
        wqkp = pool("wqkp", 1)
        wvp = pool("wvp", 1)
        woutp = pool("woutp", 1)
        xtp = pool("xt", 2)
        qp = pool("qp", 2)
        yTp = pool("yTp", 2)
        expp = pool("expp", 3)
        nrm_b = pool("nrm_b", 1)
        y8p = pool("y8", 1)
        otp = pool("ot", 1)
        psProj = pool("psProj", 2, space="PSUM")
        psS = pool("psS", 3, space="PSUM")
        psY = pool("psY", 1, space="PSUM")
        psO = pool("psO", 1, space="PSUM")

        k_sb = kp.tile([P, MQ, T], F32R)
        v_sb = vp.tile([P, TT, HL, D + 1], F32R)
        wqk_sb = wqkp.tile([P, NCK, 2 * CL], F32R)
        wv_sb = wvp.tile([P, NCK, CL], F32R)
        wout_sb = woutp.tile([P, YC, C], F32R)
        bqk_sb = const.tile([P, MQK], F32)
        bv_sb = const.tile([1, CL], F32R)
        mask01 = const.tile([P, P], F32)
        onescr = const.tile([P, TT * HL], F32)

        nc.sync.dma_start(out=bqk_sb[:, :], in_=bqk[:, :])
        nc.sync.dma_start(out=bv_sb[:, :], in_=bv[:, :])
        for c in range(NCK):
            nc.sync.dma_start(out=wqk_sb[:, c, :], in_=wqk[c * P:(c + 1) * P, :])
            nc.sync.dma_start(out=wv_sb[:, c, :], in_=wv[c * P:(c + 1) * P, :])
        for c in range(YC):
            nc.sync.dma_start(out=wout_sb[:, c, :], in_=wout[c * P:(c + 1) * P, :])
        # mask01[p, f] = 1 if f >= p else 0  (S^T visibility: tq >= tk).
        # mask01 feeds only the GpSimd multiply (not a matmul), so plain f32.
        make_upper_triangular(nc, mask01[:, :], val=1.0, diag=True)
        # f32r tiles can't be memset; memset f32 scratch + DVE copy (rounds)
        nc.vector.memset(onescr[:, :], 1.0)
        nc.vector.tensor_copy(
            v_sb[:, :, :, D],
            onescr[:, :].rearrange("p (t h) -> p t h", h=HL))
        ones1 = v_sb[0:1, :, :, D].rearrange("u t h -> u (t h)")

        def body():
            for s in range(NS):
                t0 = s * SLAB
                # ---- A(s): projections for this slab ----
                xt = xtp.tile([P, NCK, SLAB], F32R)
                for c in range(NCK):
                    nc.sync.dma_start(out=xt[:, c, :],
                                      in_=xT[c * P:(c + 1) * P, t0:t0 + SLAB])
                q_sb = qp.tile([P, MQ, SLAB], F32R)
                for m in range(MQK if "q" in parts else 0):
                    ps = psProj.tile([P, SLAB], F32, tag="ps")
                    for c in range(NCK):
                        nc.tensor.matmul(
                            ps[:, :],
                            wqk_sb[:, c, m * P:(m + 1) * P],
                            xt[:, c, :],
                            start=(c == 0), stop=(c == NCK - 1))
                    dst = (q_sb[:, m, :] if m < MQ
                           else k_sb[:, m - MQ, t0:t0 + SLAB])
                    sc = scale if m < MQ else 1.0
                    nc.vector.tensor_scalar(
                        dst, ps[:, :], sc, bqk_sb[:, m:m + 1],
                        op0=ALU.mult, op1=ALU.add)
                for sub in range(SLAB // P if "w" in parts else 0):
                    tt = s * (SLAB // P) + sub
                    ps = psProj.tile([P, CL], F32, tag="ps")
                    for c in range(NCK):
                        nc.tensor.matmul(
                            ps[:, :],
                            xt[:, c, sub * P:(sub + 1) * P],
                            wv_sb[:, c, :],
                            start=(c == 0), stop=False)
                    nc.tensor.matmul(
                        ps[:, :], ones1[:, :],
                        bv_sb[0:1, :], start=False, stop=True)
                    nc.vector.tensor_copy(
                        v_sb[:, tt, :, 0:D],
                        ps[:, :].rearrange("p (h d) -> p h d", d=D))

                # ---- B: attention; even/odd head pairs share the PE
                # array via tile_position row groups (concurrent K=64) ----
                yT_sb = yTp.tile([P, YC, SLAB], F32R)
                for hp in range(HL // 2 if "b" in parts else 0):
                    nblk = (s + 1) * SLAB // P
                    py0 = psY.tile([D + 1, SLAB], F32, tag="py0")
                    py1 = psY.tile([D + 1, SLAB], F32, tag="py1")
                    pys = (py0, py1)
                    for b in range(nblk):
                        tk0 = b * P
                        off = tk0 - t0
                        vis = max(0, off)
                        pss = []
                        for i in range(2):
                            row0 = i * 64
                            ps = psS.tile([P, SLAB], F32)
                            nc.tensor.matmul(
                                ps[:, vis:SLAB],
                                k_sb[row0:row0 + 64, hp, tk0:tk0 + P],
                                q_sb[row0:row0 + 64, hp, vis:SLAB],
                                start=True, stop=True,
                                tile_position=(row0, 0))
                            pss.append(ps)
                        eps = []
                        for i in range(2):
                            ep = expp.tile([P, SLAB], F32R)
                            nc.scalar.activation(ep[:, vis:SLAB],
                                                 pss[i][:, vis:SLAB], AF.Exp)
                            if off >= 0:
                                nc.gpsimd.tensor_mul(
                                    ep[:, off:off + P], ep[:, off:off + P],
                                    mask01[:, :])
                            eps.append(ep)
                        for i in range(2):
                            nc.tensor.matmul(
                                pys[i][0:D + 1, vis:SLAB],
                                v_sb[:, b, 2 * hp + i, 0:D + 1],
                                eps[i][:, vis:SLAB],
                                start=(b == 0), stop=(b == nblk - 1))
                    for i in range(2):
                        h = 2 * hp + i
                        row0 = i * 64
                        bi = nrm_b.tile([64, SLAB], F32, tag="binv")
                        nc.vector.reciprocal(bi[0:1, :], pys[i][D:D + 1, :])
                        sidx = h * NS + s
                        nc.sync.dma_start(out=scr[sidx:sidx + 1, :],
                                          in_=bi[0:1, :])
                        src = scr[sidx:sidx + 1, :]
                        bsrc = bass.AP(tensor=src.tensor, offset=src.offset,
                                       ap=[[0, 64], [1, SLAB]])
                        nc.sync.dma_start(out=bi[:, :], in_=bsrc)
                        y8 = y8p.tile([64, SLAB], F32R)
                        nc.vector.tensor_mul(y8[:, :], pys[i][0:D, :], bi[:, :])
                        nc.sync.dma_start(
                            out=yT_sb[row0:row0 + 64, hp, :], in_=y8[:, :])

                # ---- C(s): out projection for this slab ----
                for sub in range(SLAB // P if "c" in parts else 0):
                    for n in range(NOUT):
                        n0 = n * W_OUT
                        ps = psO.tile([P, W_OUT], F32)
                        for c in range(YC):
                            nc.tensor.matmul(
                                ps[:, :],
                                yT_sb[:, c, sub * P:(sub + 1) * P],
                                wout_sb[:, c, n0:n0 + W_OUT],
                                start=(c == 0), stop=(c == YC - 1))
                        ot = otp.tile([P, W_OUT], F32)
                        nc.vector.tensor_copy(ot[:, :], ps[:, :])
                        nc.sync.dma_start(
                            out=outp[t0 + sub * P:t0 + (sub + 1) * P,
                                     n0:n0 + W_OUT],
                            in_=ot[:, :])

        if loop_reps is None:
            body()
        else:
            with tc.For_i(0, loop_reps, 1):
                body()

    nc.compile()
    return nc


_NC_CACHE = None


def _get_nc():
    global _NC_CACHE
    if _NC_CACHE is None:
        _NC_CACHE = _build_nc()
    return _NC_CACHE


def make_in_maps(x, W_qkv, b_qkv, W_out):
    scale = 1.0 / np.sqrt(D)
    MQK = 2 * CL // P
    in_maps = []
    for core in range(N_CORES):
        b, hg = divmod(core, N_GROUPS)
        qs = slice(hg * CL, (hg + 1) * CL)
        ks = slice(C + hg * CL, C + (hg + 1) * CL)
        vs = slice(2 * C + hg * CL, 2 * C + (hg + 1) * CL)
        bqk_cat = np.concatenate([b_qkv[qs] * scale, b_qkv[ks]])
        in_maps.append({
            "xT": np.ascontiguousarray(x[b].T),
            "wqk": np.ascontiguousarray(
                np.concatenate([W_qkv[:, qs], W_qkv[:, ks]], axis=1)),
            "wv": np.ascontiguousarray(W_qkv[:, vs]),
            "wout": np.ascontiguousarray(W_out[hg * CL:(hg + 1) * CL, :]),
            "bqk": np.ascontiguousarray(bqk_cat.reshape(MQK, P).T),
            "bv": np.ascontiguousarray(b_qkv[vs].reshape(1, CL)),
        })
    return in_maps


def kernel(x, W_qkv, b_qkv, W_out, b_out):
    x = np.asarray(x, dtype=np.float32)
    W_qkv = np.asarray(W_qkv, dtype=np.float32)
    b_qkv = np.asarray(b_qkv, dtype=np.float32)
    W_out = np.asarray(W_out, dtype=np.float32)
    b_out = np.asarray(b_out, dtype=np.float32)

    nc = _get_nc()
    in_maps = make_in_maps(x, W_qkv, b_qkv, W_out)
    res = run_bass_kernel_spmd(nc, in_maps, core_ids=list(range(N_CORES)))

    out = np.empty((B, T, C), dtype=np.float32)
    for b in range(B):
        out[b] = (res.results[N_GROUPS * b]["outp"]
                  + res.results[N_GROUPS * b + 1]["outp"] + b_out)
    return out



# revision 4
# speedup vs baseline: 1.3173x; 1.3173x over previous
"""Causal self-attention (B=4, T=2048, C=1024, H=16) on 8 TRN2 NeuronCores.

Sharding: core = (batch, head-group) — data parallel over the 4 batches,
tensor parallel over 2 groups of 8 heads (Megatron-style column/row split of
the qkv / out projections).  Each core computes a [T, C] partial of the out
projection for its head group; the host sums the two partials per batch and
adds b_out, so no device collectives are needed.

v2 (vs v1 baseline at ~630us):
  * All matmul operands are bf16 (PSUM accumulation stays fp32).  Same PE
    streaming rate as f32r (1 cycle/row) but enables Fast Weight Load
    (v1 spent 253us in serialized fp32 LDWEIGHTS), removes the f32r
    moving-dim<256 4x penalty, and halves DMA + SBUF footprint.  Host casts
    inputs to bf16.
  * One exp ACTIVATE per tk-block covering BOTH heads of a pair via a
    [128, 2, 512] PSUM tile spanning 2 banks (v1: 320 exps -> 160, less
    fixed per-instruction overhead on ScalarE).
  * Softmax normalization: v1 burned 107us of DVE in single-partition
    5-pass RECIPROCALs.  Now: copy the two denominator rows (PSUM row D)
    to partitions 0/1, one reciprocal_approx_fast on [2,512], one DRAM
    bounce DMA broadcasting both heads' 1/den to [64, 2, 512], then one
    tensor_mul per head.
  * Loop body stays slab-interleaved (projections / attention / out-proj)
    so the Tile scheduler can fill PE gaps during ScalarE exp latency with
    next-slab projection matmuls — keeping the PE HAM-warm at 2.4 GHz
    (v1 ran 67% of the time at the 1.2 GHz throttle).
"""

import os
import sys
from contextlib import ExitStack

import numpy as np

for _p in ("/opt/trn_rl_repo", "/root/.axon_site/_ro/trn_rl_repo"):
    if os.path.isdir(_p) and _p not in sys.path:
        sys.path.append(_p)

import concourse.bacc as bacc
import concourse.bass as bass
import concourse.tile as tile
from concourse import mybir
from concourse.bass_utils import run_bass_kernel_spmd
from concourse.masks import make_upper_triangular

AF = mybir.ActivationFunctionType
ALU = mybir.AluOpType
F32 = mybir.dt.float32
BF16 = mybir.dt.bfloat16

P = 128
SLAB = 512

B, T, C, H, D = 4, 2048, 1024, 16, 64
N_CORES = 8
N_GROUPS = 2          # head groups (tensor-parallel degree per batch)
HL = H // N_GROUPS    # heads per core
CL = HL * D           # local qkv width


def _build_nc():
    NCK = C // P
    MQK = 2 * CL // P
    MQ = MQK // 2
    TT = T // P
    NS = T // SLAB
    YC = CL // P
    W_OUT = min(SLAB, C)
    NOUT = C // W_OUT
    scale = 1.0 / np.sqrt(D)

    nc = bacc.Bacc("TRN2", target_bir_lowering=False, debug=False,
                   num_devices=N_CORES)
    xT = nc.dram_tensor("xT", [C, T], BF16, kind="ExternalInput")
    wqk = nc.dram_tensor("wqk", [C, 2 * CL], BF16, kind="ExternalInput")
    wv = nc.dram_tensor("wv", [C, CL], BF16, kind="ExternalInput")
    wout = nc.dram_tensor("wout", [CL, C], BF16, kind="ExternalInput")
    bqk = nc.dram_tensor("bqk", [P, MQK], F32, kind="ExternalInput")
    bv = nc.dram_tensor("bv", [1, CL], BF16, kind="ExternalInput")
    outp = nc.dram_tensor("outp", [T, C], BF16, kind="ExternalOutput")
    scr = nc.dram_tensor("scr", [2 * HL // 2 * NS, SLAB], F32)

    with tile.TileContext(nc) as tc, ExitStack() as ctx:
        pool = lambda name, bufs, **kw: ctx.enter_context(
            tc.tile_pool(name=name, bufs=bufs, **kw))

        const = pool("const", 1)
        kp = pool("kp", 1)
        vp = pool("vp", 1)
        wqkp = pool("wqkp", 1)
        wvp = pool("wvp", 1)
        woutp = pool("woutp", 1)
        xtp = pool("xt", 2)
        qp = pool("qp", 2)
        yTp = pool("yTp", 2)
        expp = pool("expp", 3)
        den2p = pool("den2p", 2)
        recbp = pool("recbp", 2)
        y8p = pool("y8", 2)
        otp = pool("ot", 2)
        psS = pool("psS", 2, space="PSUM")      # [P,2,SLAB] pair tiles: 4 banks
        psY = pool("psY", 1, space="PSUM")      # py0+py1: 2 banks
        psPO = pool("psPO", 2, space="PSUM")    # shared proj/out evict: 2 banks

        k_sb = kp.tile([P, MQ, T], BF16)
        v_sb = vp.tile([P, TT, HL, D + 1], BF16)
        wqk_sb = wqkp.tile([P, NCK, 2 * CL], BF16)
        wv_sb = wvp.tile([P, NCK, CL], BF16)
        wout_sb = woutp.tile([P, YC, C], BF16)
        bqk_sb = const.tile([P, MQK], F32)
        bv_sb = const.tile([1, CL], BF16)
        mask01 = const.tile([P, P], BF16)
        maskf = const.tile([P, P], F32)
        ones_row = const.tile([1, P], BF16)

        for c in range(NCK):
            nc.sync.dma_start(out=wqk_sb[:, c, :], in_=wqk[c * P:(c + 1) * P, :])
        nc.sync.dma_start(out=bqk_sb[:, :], in_=bqk[:, :])
        nc.sync.dma_start(out=bv_sb[:, :], in_=bv[:, :])
        for c in range(NCK):
            nc.sync.dma_start(out=wv_sb[:, c, :], in_=wv[c * P:(c + 1) * P, :])
        for c in range(YC):
            nc.sync.dma_start(out=wout_sb[:, c, :], in_=wout[c * P:(c + 1) * P, :])
        # mask01[p, f] = 1 if f >= p else 0  (S^T visibility: tq >= tk);
        # built in f32 then cast (affine_select path is f32).
        make_upper_triangular(nc, maskf[:, :], val=1.0, diag=True)
        nc.vector.tensor_copy(mask01[:, :], maskf[:, :])
        nc.vector.memset(ones_row[:, :], 1.0)
        nc.vector.memset(v_sb[:, :, :, D], 1.0)

        for s in range(NS):
            t0 = s * SLAB
            # ---- A(s): projections for this slab ----
            xt = xtp.tile([P, NCK, SLAB], BF16)
            for c in range(NCK):
                nc.sync.dma_start(out=xt[:, c, :],
                                  in_=xT[c * P:(c + 1) * P, t0:t0 + SLAB])
            q_sb = qp.tile([P, MQ, SLAB], BF16)
            for m in range(MQK):
                ps = psPO.tile([P, SLAB], F32, tag="po")
                for c in range(NCK):
                    nc.tensor.matmul(
                        ps[:, :],
                        wqk_sb[:, c, m * P:(m + 1) * P],
                        xt[:, c, :],
                        start=(c == 0), stop=(c == NCK - 1))
                dst = (q_sb[:, m, :] if m < MQ
                       else k_sb[:, m - MQ, t0:t0 + SLAB])
                sc = scale if m < MQ else 1.0
                nc.vector.tensor_scalar(
                    dst, ps[:, :], sc, bqk_sb[:, m:m + 1],
                    op0=ALU.mult, op1=ALU.add)
            for sub in range(SLAB // P):
                tt = s * (SLAB // P) + sub
                ps = psPO.tile([P, CL], F32, tag="po")
                for c in range(NCK):
                    nc.tensor.matmul(
                        ps[:, :],
                        xt[:, c, sub * P:(sub + 1) * P],
                        wv_sb[:, c, :],
                        start=(c == 0), stop=False)
                nc.tensor.matmul(
                    ps[:, :], ones_row[:, :],
                    bv_sb[0:1, :], start=False, stop=True)
                nc.vector.tensor_copy(
                    v_sb[:, tt, :, 0:D],
                    ps[:, :].rearrange("p (h d) -> p h d", d=D))

            # ---- B(s): attention; even/odd head pairs share the PE array
            # via tile_position row groups (concurrent K=64).  One exp per
            # tk-block covers both heads ([P, 2, SLAB] across 2 PSUM banks).
            yT_sb = yTp.tile([P, YC, SLAB], BF16)
            for hp in range(HL // 2):
                nblk = (s + 1) * SLAB // P
                py0 = psY.tile([D + 1, SLAB], F32, tag="py0")
                py1 = psY.tile([D + 1, SLAB], F32, tag="py1")
                pys = (py0, py1)
                for b in range(nblk):
                    tk0 = b * P
                    off = tk0 - t0
                    vis = max(0, off)
                    ps3 = psS.tile([P, 2, SLAB], F32)
                    for i in range(2):
                        row0 = i * 64
                        nc.tensor.matmul(
                            ps3[:, i, vis:SLAB],
                            k_sb[row0:row0 + 64, hp, tk0:tk0 + P],
                            q_sb[row0:row0 + 64, hp, vis:SLAB],
                            start=True, stop=True,
                            tile_position=(row0, 0))
                    ep3 = expp.tile([P, 2, SLAB], BF16)
                    nc.scalar.activation(ep3[:, :, vis:SLAB],
                                         ps3[:, :, vis:SLAB], AF.Exp)
                    if off >= 0:
                        for i in range(2):
                            nc.gpsimd.tensor_mul(
                                ep3[:, i, off:off + P],
                                ep3[:, i, off:off + P], mask01[:, :])
                    for i in range(2):
                        nc.tensor.matmul(
                            pys[i][0:D + 1, vis:SLAB],
                            v_sb[:, b, 2 * hp + i, 0:D + 1],
                            ep3[:, i, vis:SLAB],
                            start=(b == 0), stop=(b == nblk - 1))
                # normalize both heads: 1/den broadcast via one DRAM bounce.
                # (reciprocal_approx_fast needs an SBUF source — PSUM reads
                # feed it garbage — so stage the two den rows into SBUF.)
                sidx = 2 * (hp * NS + s)
                den2 = den2p.tile([1, 2 * SLAB], F32)
                rec2 = den2p.tile([1, 2 * SLAB], F32, tag="rec2")
                for i in range(2):
                    nc.scalar.copy(den2[:, i * SLAB:(i + 1) * SLAB],
                                   pys[i][D:D + 1, :])
                nc.vector.reciprocal_approx_fast(rec2[:, :], den2[:, :])
                nc.sync.dma_start(out=scr[sidx:sidx + 2, :], in_=rec2[:, :])
                src = scr[sidx:sidx + 2, :]
                bsrc = bass.AP(tensor=src.tensor, offset=src.offset,
                               ap=[[0, 64], [SLAB, 2], [1, SLAB]])
                recb = recbp.tile([64, 2, SLAB], F32)
                nc.sync.dma_start(out=recb[:, :, :], in_=bsrc)
                for i in range(2):
                    row0 = i * 64
                    y8 = y8p.tile([64, SLAB], BF16)
                    nc.vector.tensor_mul(y8[:, :], pys[i][0:D, :],
                                         recb[:, i, :])
                    nc.sync.dma_start(
                        out=yT_sb[row0:row0 + 64, hp, :], in_=y8[:, :])

            # ---- C(s): out projection for this slab ----
            for sub in range(SLAB // P):
                for n in range(NOUT):
                    n0 = n * W_OUT
                    ps = psPO.tile([P, W_OUT], F32, tag="po")
                    for c in range(YC):
                        nc.tensor.matmul(
                            ps[:, :],
                            yT_sb[:, c, sub * P:(sub + 1) * P],
                            wout_sb[:, c, n0:n0 + W_OUT],
                            start=(c == 0), stop=(c == YC - 1))
                    ot = otp.tile([P, W_OUT], BF16)
                    nc.vector.tensor_copy(ot[:, :], ps[:, :])
                    nc.sync.dma_start(
                        out=outp[t0 + sub * P:t0 + (sub + 1) * P,
                                 n0:n0 + W_OUT],
                        in_=ot[:, :])

    nc.compile()
    return nc


_NC_CACHE = None


def _get_nc():
    global _NC_CACHE
    if _NC_CACHE is None:
        _NC_CACHE = _build_nc()
    return _NC_CACHE


def make_in_maps(x, W_qkv, b_qkv, W_out):
    bf16 = mybir.dt.np(BF16)
    scale = 1.0 / np.sqrt(D)
    MQK = 2 * CL // P
    in_maps = []
    for core in range(N_CORES):
        b, hg = divmod(core, N_GROUPS)
        qs = slice(hg * CL, (hg + 1) * CL)
        ks = slice(C + hg * CL, C + (hg + 1) * CL)
        vs = slice(2 * C + hg * CL, 2 * C + (hg + 1) * CL)
        bqk_cat = np.concatenate([b_qkv[qs] * scale, b_qkv[ks]])
        in_maps.append({
            "xT": np.ascontiguousarray(x[b].T).astype(bf16),
            "wqk": np.ascontiguousarray(
                np.concatenate([W_qkv[:, qs], W_qkv[:, ks]],
                               axis=1)).astype(bf16),
            "wv": np.ascontiguousarray(W_qkv[:, vs]).astype(bf16),
            "wout": np.ascontiguousarray(W_out[hg * CL:(hg + 1) * CL,
                                               :]).astype(bf16),
            "bqk": np.ascontiguousarray(bqk_cat.reshape(MQK, P).T),
            "bv": np.ascontiguousarray(b_qkv[vs].reshape(1, CL)).astype(bf16),
        })
    return in_maps


def kernel(x, W_qkv, b_qkv, W_out, b_out):
    x = np.asarray(x, dtype=np.float32)
    W_qkv = np.asarray(W_qkv, dtype=np.float32)
    b_qkv = np.asarray(b_qkv, dtype=np.float32)
    W_out = np.asarray(W_out, dtype=np.float32)
    b_out = np.asarray(b_out, dtype=np.float32)

    nc = _get_nc()
    in_maps = make_in_maps(x, W_qkv, b_qkv, W_out)
    res = run_bass_kernel_spmd(nc, in_maps, core_ids=list(range(N_CORES)))

    out = np.empty((B, T, C), dtype=np.float32)
    for b in range(B):
        out[b] = (res.results[N_GROUPS * b]["outp"].astype(np.float32)
                  + res.results[N_GROUPS * b + 1]["outp"].astype(np.float32)
                  + b_out)
    return out


# revision 11
# speedup vs baseline: 1.6358x; 1.2417x over previous
"""Causal self-attention (B=4, T=2048, C=1024, H=16) on 8 TRN2 NeuronCores.

Sharding: core = (batch, head-group) — data parallel over the 4 batches,
tensor parallel over 2 groups of 8 heads (Megatron-style column/row split of
the qkv / out projections).  Each core computes a [T, C] partial of the out
projection for its head group; the host sums the two partials per batch and
adds b_out, so no device collectives are needed.

v2 (vs v1 baseline at ~630us):
  * All matmul operands are bf16 (PSUM accumulation stays fp32).  Same PE
    streaming rate as f32r (1 cycle/row) but enables Fast Weight Load
    (v1 spent 253us in serialized fp32 LDWEIGHTS), removes the f32r
    moving-dim<256 4x penalty, and halves DMA + SBUF footprint.  Host casts
    inputs to bf16.
  * One exp ACTIVATE per tk-block covering BOTH heads of a pair via a
    [128, 2, 512] PSUM tile spanning 2 banks (v1: 320 exps -> 160, less
    fixed per-instruction overhead on ScalarE).
  * Softmax normalization: v1 burned 107us of DVE in single-partition
    5-pass RECIPROCALs.  Now: copy the two denominator rows (PSUM row D)
    to partitions 0/1, one reciprocal_approx_fast on [2,512], one DRAM
    bounce DMA broadcasting both heads' 1/den to [64, 2, 512], then one
    tensor_mul per head.
  * Loop body stays slab-interleaved (projections / attention / out-proj)
    so the Tile scheduler can fill PE gaps during ScalarE exp latency with
    next-slab projection matmuls — keeping the PE HAM-warm at 2.4 GHz
    (v1 ran 67% of the time at the 1.2 GHz throttle).
"""

import os
import sys
from contextlib import ExitStack

import numpy as np

for _p in ("/opt/trn_rl_repo", "/root/.axon_site/_ro/trn_rl_repo"):
    if os.path.isdir(_p) and _p not in sys.path:
        sys.path.append(_p)

import concourse.bacc as bacc
import concourse.bass as bass
import concourse.tile as tile
from concourse import mybir
from concourse.bass_utils import run_bass_kernel_spmd
from concourse.masks import make_upper_triangular

AF = mybir.ActivationFunctionType
ALU = mybir.AluOpType
F32 = mybir.dt.float32
BF16 = mybir.dt.bfloat16

P = 128
SLAB = 512

B, T, C, H, D = 4, 2048, 1024, 16, 64
N_CORES = 8
N_GROUPS = 2          # head groups (tensor-parallel degree per batch)
HL = H // N_GROUPS    # heads per core
CL = HL * D           # local qkv width


def _build_nc():
    NCK = C // P
    MQK = 2 * CL // P
    MQ = MQK // 2
    TT = T // P
    NS = T // SLAB
    YC = CL // P
    W_OUT = min(SLAB, C)
    NOUT = C // W_OUT
    scale = 1.0 / np.sqrt(D)

    nc = bacc.Bacc("TRN2", target_bir_lowering=False, debug=False,
                   num_devices=N_CORES)
    xT = nc.dram_tensor("xT", [C, T], BF16, kind="ExternalInput")
    wqk = nc.dram_tensor("wqk", [C, 2 * CL], BF16, kind="ExternalInput")
    wv = nc.dram_tensor("wv", [C, CL], BF16, kind="ExternalInput")
    wout = nc.dram_tensor("wout", [CL, C], BF16, kind="ExternalInput")
    bqk = nc.dram_tensor("bqk", [P, MQK], F32, kind="ExternalInput")
    bv = nc.dram_tensor("bv", [1, CL], BF16, kind="ExternalInput")
    outp = nc.dram_tensor("outp", [T, C], BF16, kind="ExternalOutput")
    scr = nc.dram_tensor("scr", [2 * HL // 2 * NS, SLAB], F32)

    with tile.TileContext(nc) as tc, ExitStack() as ctx:
        pool = lambda name, bufs, **kw: ctx.enter_context(
            tc.tile_pool(name=name, bufs=bufs, **kw))

        const = pool("const", 1)
        kp = pool("kp", 1)
        vp = pool("vp", 1)
        wqkp = pool("wqkp", 1)
        wvp = pool("wvp", 1)
        woutp = pool("woutp", 1)
        xtp = pool("xt", 2)
        qp = pool("qp", 2)
        yTp = pool("yTp", 2)
        expp = pool("expp", 3)
        yrawp = pool("yrawp", 2)
        den2p = pool("den2p", 2)
        recbp = pool("recbp", 2)
        y8p = pool("y8", 2)
        otp = pool("ot", 2)
        psS = pool("psS", 2, space="PSUM")      # [P,2,SLAB] pair tiles: 4 banks
        psY = pool("psY", 1, space="PSUM")      # py0+py1: 2 banks
        psPO = pool("psPO", 2, space="PSUM")    # shared proj/out evict: 2 banks

        k_sb = kp.tile([P, MQ, T], BF16)
        v_sb = vp.tile([P, TT, HL, D + 1], BF16)
        wqk_sb = wqkp.tile([P, NCK, 2 * CL], BF16)
        wv_sb = wvp.tile([P, NCK, CL], BF16)
        wout_sb = woutp.tile([P, YC, C], BF16)
        bqk_sb = const.tile([P, MQK], F32)
        bv_sb = const.tile([1, CL], BF16)
        mask01 = const.tile([P, P], BF16)
        maskf = const.tile([P, P], F32)
        ones_row = const.tile([1, P], BF16)

        # Preload: sync queue carries only what phase A(0) needs first (wqk);
        # everything else goes through the gpsimd SWDGE queue so the ~0.6us
        # per-DMA issue cost doesn't serialize ahead of the first matmuls.
        for c in range(NCK):
            nc.sync.dma_start(out=wqk_sb[:, c, :], in_=wqk[c * P:(c + 1) * P, :])
        nc.gpsimd.dma_start(out=bqk_sb[:, :], in_=bqk[:, :])
        nc.gpsimd.dma_start(out=bv_sb[:, :], in_=bv[:, :])
        for c in range(NCK):
            nc.gpsimd.dma_start(out=wv_sb[:, c, :], in_=wv[c * P:(c + 1) * P, :])
        for c in range(YC):
            nc.gpsimd.dma_start(out=wout_sb[:, c, :],
                                in_=wout[c * P:(c + 1) * P, :])
        # mask01[p, f] = 1 if f >= p else 0  (S^T visibility: tq >= tk);
        # built in f32 then cast (affine_select path is f32).
        make_upper_triangular(nc, maskf[:, :], val=1.0, diag=True)
        nc.vector.tensor_copy(mask01[:, :], maskf[:, :])
        nc.vector.memset(ones_row[:, :], 1.0)
        nc.vector.memset(v_sb[:, :, :, D], 1.0)

        def fetch_xt(s):
            t0 = s * SLAB
            xt = xtp.tile([P, NCK, SLAB], BF16, tag="xt")
            for c in range(NCK):
                nc.sync.dma_start(out=xt[:, c, :],
                                  in_=xT[c * P:(c + 1) * P, t0:t0 + SLAB])
            return xt

        xt_next = fetch_xt(0)
        for s in range(NS):
            t0 = s * SLAB
            # ---- A(s): projections for this slab ----
            xt = xt_next
            q_sb = qp.tile([P, MQ, SLAB], BF16)
            for m in range(MQK):
                ps = psPO.tile([P, SLAB], F32, tag="po")
                for c in range(NCK):
                    nc.tensor.matmul(
                        ps[:, :],
                        wqk_sb[:, c, m * P:(m + 1) * P],
                        xt[:, c, :],
                        start=(c == 0), stop=(c == NCK - 1))
                dst = (q_sb[:, m, :] if m < MQ
                       else k_sb[:, m - MQ, t0:t0 + SLAB])
                sc = scale if m < MQ else 1.0
                nc.vector.tensor_scalar(
                    dst, ps[:, :], sc, bqk_sb[:, m:m + 1],
                    op0=ALU.mult, op1=ALU.add)
            for sub in range(SLAB // P):
                tt = s * (SLAB // P) + sub
                ps = psPO.tile([P, CL], F32, tag="po")
                for c in range(NCK):
                    nc.tensor.matmul(
                        ps[:, :],
                        xt[:, c, sub * P:(sub + 1) * P],
                        wv_sb[:, c, :],
                        start=(c == 0), stop=False)
                nc.tensor.matmul(
                    ps[:, :], ones_row[:, :],
                    bv_sb[0:1, :], start=False, stop=True)
                nc.vector.tensor_copy(
                    v_sb[:, tt, :, 0:D],
                    ps[:, :].rearrange("p (h d) -> p h d", d=D))

            # Prefetch next slab's x BEFORE B(s) is emitted: the DMA queues
            # are FIFO in priority order, so emitting these after B(s)'s
            # yT/bounce DMAs would head-of-line-block them behind attention.
            if s + 1 < NS:
                xt_next = fetch_xt(s + 1)

            # ---- B(s): attention; even/odd head pairs share the PE array
            # via tile_position row groups (concurrent K=64).  One exp per
            # tk-block covers both heads ([P, 2, SLAB] across 2 PSUM banks).
            yT_sb = yTp.tile([P, YC, SLAB], BF16)
            for hp in range(HL // 2):
                nblk = (s + 1) * SLAB // P
                py0 = psY.tile([D + 1, SLAB], F32, tag="py0")
                py1 = psY.tile([D + 1, SLAB], F32, tag="py1")
                pys = (py0, py1)
                for b in range(nblk):
                    tk0 = b * P
                    off = tk0 - t0
                    vis = max(0, off)
                    ps3 = psS.tile([P, 2, SLAB], F32)
                    for i in range(2):
                        row0 = i * 64
                        nc.tensor.matmul(
                            ps3[:, i, vis:SLAB],
                            k_sb[row0:row0 + 64, hp, tk0:tk0 + P],
                            q_sb[row0:row0 + 64, hp, vis:SLAB],
                            start=True, stop=True,
                            tile_position=(row0, 0))
                    ep3 = expp.tile([P, 2, SLAB], BF16)
                    nc.scalar.activation(ep3[:, :, vis:SLAB],
                                         ps3[:, :, vis:SLAB], AF.Exp)
                    if off >= 0:
                        for i in range(2):
                            nc.gpsimd.tensor_mul(
                                ep3[:, i, off:off + P],
                                ep3[:, i, off:off + P], mask01[:, :])
                    for i in range(2):
                        nc.tensor.matmul(
                            pys[i][0:D + 1, vis:SLAB],
                            v_sb[:, b, 2 * hp + i, 0:D + 1],
                            ep3[:, i, vis:SLAB],
                            start=(b == 0), stop=(b == nblk - 1))
                # Evict py0/py1 RAW to SBUF immediately (ScalarE + DVE in
                # parallel) so the psY banks free up and the next pair's PV
                # can start; the whole normalize chain then runs from SBUF
                # off the PE critical path.  (reciprocal_approx_fast needs
                # an SBUF source — PSUM reads feed it garbage.)
                yraw0 = yrawp.tile([D + 1, SLAB], F32, tag="yraw0")
                yraw1 = yrawp.tile([D + 1, SLAB], F32, tag="yraw1")
                nc.scalar.copy(yraw0[:, :], py0[:, :])
                nc.vector.tensor_copy(yraw1[:, :], py1[:, :])
                yraws = (yraw0, yraw1)
                sidx = 2 * (hp * NS + s)
                # Bounce the RAW denominator rows through DRAM to broadcast
                # them across partitions, then reciprocal on the broadcast
                # tile (SBUF, partition base 0 — the only layout the custom
                # DVE reciprocal handles).
                nc.sync.dma_start(out=scr[sidx:sidx + 1, :],
                                  in_=yraw0[D:D + 1, :])
                nc.sync.dma_start(out=scr[sidx + 1:sidx + 2, :],
                                  in_=yraw1[D:D + 1, :])
                src = scr[sidx:sidx + 2, :]
                bsrc = bass.AP(tensor=src.tensor, offset=src.offset,
                               ap=[[0, 64], [SLAB, 2], [1, SLAB]])
                denb = recbp.tile([64, 2, SLAB], F32, tag="denb")
                recb = recbp.tile([64, 2, SLAB], F32, tag="recb")
                nc.sync.dma_start(out=denb[:, :, :], in_=bsrc)
                nc.vector.reciprocal_approx_fast(recb[:, :, :],
                                                 denb[:, :, :])
                for i in range(2):
                    row0 = i * 64
                    y8 = y8p.tile([64, SLAB], BF16)
                    nc.vector.tensor_mul(y8[:, :], yraws[i][0:D, :],
                                         recb[:, i, :])
                    nc.sync.dma_start(
                        out=yT_sb[row0:row0 + 64, hp, :], in_=y8[:, :])

            # ---- C(s): out projection for this slab ----
            for sub in range(SLAB // P):
                for n in range(NOUT):
                    n0 = n * W_OUT
                    ps = psPO.tile([P, W_OUT], F32, tag="po")
                    for c in range(YC):
                        nc.tensor.matmul(
                            ps[:, :],
                            yT_sb[:, c, sub * P:(sub + 1) * P],
                            wout_sb[:, c, n0:n0 + W_OUT],
                            start=(c == 0), stop=(c == YC - 1))
                    ot = otp.tile([P, W_OUT], BF16)
                    nc.vector.tensor_copy(ot[:, :], ps[:, :])
                    nc.sync.dma_start(
                        out=outp[t0 + sub * P:t0 + (sub + 1) * P,
                                 n0:n0 + W_OUT],
                        in_=ot[:, :])

    nc.compile()
    return nc


_NC_CACHE = None


def _get_nc():
    global _NC_CACHE
    if _NC_CACHE is None:
        _NC_CACHE = _build_nc()
    return _NC_CACHE


def make_in_maps(x, W_qkv, b_qkv, W_out):
    bf16 = mybir.dt.np(BF16)
    scale = 1.0 / np.sqrt(D)
    MQK = 2 * CL // P
    in_maps = []
    for core in range(N_CORES):
        b, hg = divmod(core, N_GROUPS)
        qs = slice(hg * CL, (hg + 1) * CL)
        ks = slice(C + hg * CL, C + (hg + 1) * CL)
        vs = slice(2 * C + hg * CL, 2 * C + (hg + 1) * CL)
        bqk_cat = np.concatenate([b_qkv[qs] * scale, b_qkv[ks]])
        in_maps.append({
            "xT": np.ascontiguousarray(x[b].T).astype(bf16),
            "wqk": np.ascontiguousarray(
                np.concatenate([W_qkv[:, qs], W_qkv[:, ks]],
                               axis=1)).astype(bf16),
            "wv": np.ascontiguousarray(W_qkv[:, vs]).astype(bf16),
            "wout": np.ascontiguousarray(W_out[hg * CL:(hg + 1) * CL,
                                               :]).astype(bf16),
            "bqk": np.ascontiguousarray(bqk_cat.reshape(MQK, P).T),
            "bv": np.ascontiguousarray(b_qkv[vs].reshape(1, CL)).astype(bf16),
        })
    return in_maps


def kernel(x, W_qkv, b_qkv, W_out, b_out):
    x = np.asarray(x, dtype=np.float32)
    W_qkv = np.asarray(W_qkv, dtype=np.float32)
    b_qkv = np.asarray(b_qkv, dtype=np.float32)
    W_out = np.asarray(W_out, dtype=np.float32)
    b_out = np.asarray(b_out, dtype=np.float32)

    nc = _get_nc()
    in_maps = make_in_maps(x, W_qkv, b_qkv, W_out)
    res = run_bass_kernel_spmd(nc, in_maps, core_ids=list(range(N_CORES)))

    out = np.empty((B, T, C), dtype=np.float32)
    for b in range(B):
        out[b] = (res.results[N_GROUPS * b]["outp"].astype(np.float32)
                  + res.results[N_GROUPS * b + 1]["outp"].astype(np.float32)
                  + b_out)
    return out
